# revision 1
# baseline (speedup 1.0000x reference)
"""TRN2 Bass kernel for nn_ClassAttention (1x1 conv + BN + ReLU + windowed attention).

kernel(**inputs) takes FULL inputs, returns the FULL output [4,256,256,256] f32.
Shards data-parallel over (batch, image-row-half) across 8 NeuronCores, runs a
Bass/Tile SPMD program via run_bass_kernel_spmd, and unshards on the host.

Per-core shard (core = (b, rh) = (core//2, core%2)):
  x_sh   [256c, 16hh, 2048]   x[b,:,128rh:+128,:] rearranged window-contiguous:
                              [c, hh, (pw, win, r1, r2)]
  at_sh  [16hh, 128, 16384]   attn pre-transposed [pair, 64*win+k, 64*nh+q],
                              stored partition-major per row of windows
  w_prep [256c, 256o]         (w_conv * inv_std[:,None]).T  (BN scale folded)
  bias   [128, 256]           (beta - mean*inv_std) broadcast over partitions
  out    [16hh, 128p, 4096]   raw staging dump; host decodes
                              p = 32q+16win+d, f = u*1024+r1*128+pw*8+r2,
                              ch = 64u+16q+d

On-chip pipeline per window-pair (2 windows of 64 pixels, pixels on partitions):
  conv (PE): psum[128pix=(win,r1,r2), 256ch] = x_pair.T @ w_prep
             2 matmuls (K=128 halves), M=128, N=256, fp32
  bias (DVE): tmp = psum + bias_tile
  relu (ACT): block-diagonal V [128, (nh,win,d)]: diag cells = relu(tmp),
              off-diag cells stay zero (zeroed once at start, never rewritten)
  attn (PE): per head nh: one matmul computes BOTH windows via block-diag V:
             out[32,64] = V[:,32nh:+32].T @ At[:,64nh:+64], K=128, N=64,
             tile_position=(0, 32*(nh%4)) -> 4 column-tiles packed in the array
  evac (DVE): psum [128,(u,r1,r2)] -> staging [128, 4096]
  store (ACT hwdge ring): staging -> DRAM, 2 MiB contiguous per row of windows
"""

import numpy as np
from contextlib import ExitStack

import concourse.bacc as bacc
import concourse.tile as tile
import concourse.mybir as mybir
from concourse.bass_utils import run_bass_kernel_spmd

F32 = mybir.dt.float32
RELU = mybir.ActivationFunctionType.Relu

EPS = 1e-5
NCORES = 8

_cached_nc = None


def _build_program(n_vbd=10, at_bufs=3, G=4):
    nc = bacc.Bacc("TRN2", target_bir_lowering=False, debug=False)

    x_d = nc.dram_tensor("x_sh", [256, 16, 2048], F32, kind="ExternalInput")
    at_d = nc.dram_tensor("at_sh", [16, 128, 16384], F32, kind="ExternalInput")
    wc_d = nc.dram_tensor("w_prep", [256, 256], F32, kind="ExternalInput")
    b_d = nc.dram_tensor("bias", [128, 256], F32, kind="ExternalInput")
    out_d = nc.dram_tensor("out_sh", [16, 128, 4096], F32, kind="ExternalOutput")

    ngroups = 16 // G

    with tile.TileContext(nc) as tc, ExitStack() as ctx:
        const = ctx.enter_context(tc.tile_pool(name="const", bufs=1))
        xp = ctx.enter_context(tc.tile_pool(name="xp", bufs=2))
        atp = ctx.enter_context(tc.tile_pool(name="atp", bufs=at_bufs))
        vbdp = ctx.enter_context(tc.tile_pool(name="vbdp", bufs=1))
        tvp = ctx.enter_context(tc.tile_pool(name="tvp", bufs=4))
        stp = ctx.enter_context(tc.tile_pool(name="stp", bufs=2))
        pscp = ctx.enter_context(tc.tile_pool(name="pscp", bufs=2, space="PSUM"))
        psap = ctx.enter_context(tc.tile_pool(name="psap", bufs=4, space="PSUM"))

        w0 = const.tile([128, 256], F32, name="w0")
        w1 = const.tile([128, 256], F32, name="w1")
        nc.sync.dma_start(out=w0, in_=wc_d[0:128, :])
        nc.sync.dma_start(out=w1, in_=wc_d[128:256, :])
        bias = const.tile([128, 256], F32, name="bias_t")
        nc.sync.dma_start(out=bias, in_=b_d[:, :])

        # Block-diagonal V tiles: columns = (nh 16, win 2, d 16). Zeroed once;
        # the relu writes only the diagonal cells (win0 -> rows 0:64 of win-0
        # columns, win1 -> rows 64:128 of win-1 columns), so the zeros persist
        # across reuse and each V[:, 32nh:+32] is exactly block-diag(V0, V1).
        vbd = []
        for i in range(n_vbd):
            t = vbdp.tile([128, 512], F32, tag=f"vbd{i}", name=f"vbd{i}")
            nc.vector.memset(t, 0.0)
            vbd.append(t)
        vbd_i = 0

        for hh in range(16):
            xt0 = xp.tile([128, 2048], F32, tag="xt0", name=f"xt0_{hh}")
            xt1 = xp.tile([128, 2048], F32, tag="xt1", name=f"xt1_{hh}")
            nc.sync.dma_start(out=xt0, in_=x_d[0:128, hh, :])
            nc.sync.dma_start(out=xt1, in_=x_d[128:256, hh, :])

            st = stp.tile([128, 4096], F32, tag="st", name=f"st_{hh}")
            # f = u*1024 + r1*128 + pw*8 + r2
            st_r = st.rearrange("p (u r1 pw r2) -> p pw u r1 r2",
                                u=4, r1=8, pw=16, r2=8)

            for g in range(ngroups):
                at = atp.tile([128, 1024 * G], F32, tag="at", name=f"at_{hh}_{g}")
                nc.sync.dma_start(
                    out=at,
                    in_=at_d[hh, :, 1024 * G * g: 1024 * G * (g + 1)])

                Vg = []
                for iG in range(G):
                    p8 = G * g + iG
                    ps = pscp.tile([128, 256], F32, tag="psc", name=f"ps_{hh}_{p8}")
                    xsl = slice(128 * p8, 128 * p8 + 128)
                    nc.tensor.matmul(ps, xt0[:, xsl], w0, start=True, stop=False)
                    nc.tensor.matmul(ps, xt1[:, xsl], w1, start=False, stop=True)
                    tv = tvp.tile([128, 256], F32, tag="tv", name=f"tv_{hh}_{p8}")
                    nc.vector.tensor_add(tv, ps, bias)
                    V = vbd[vbd_i % n_vbd]
                    vbd_i += 1
                    Vr = V.rearrange("p (nh two d) -> p nh two d", nh=16, two=2, d=16)
                    tvr = tv.rearrange("p (a b) -> p a b", a=16)
                    nc.scalar.activation(Vr[0:64, :, 0, :], tvr[0:64], RELU)
                    nc.scalar.activation(Vr[64:128, :, 1, :], tvr[64:128], RELU)
                    Vg.append(V)

                for iG in range(G):
                    p8 = G * g + iG
                    V = Vg[iG]
                    pa = psap.tile([128, 256], F32, tag="pa", name=f"pa_{hh}_{p8}")
                    for j in range(4):
                        for quad in range(4):
                            nh = 4 * j + quad
                            nc.tensor.matmul(
                                pa[32 * quad:32 * quad + 32, 64 * j:64 * j + 64],
                                V[:, 32 * nh:32 * nh + 32],
                                at[:, 1024 * iG + 64 * nh: 1024 * iG + 64 * nh + 64],
                                start=True, stop=True,
                                tile_position=(0, 32 * quad))
                    src = pa.rearrange("p (u r1 r2) -> p u r1 r2", u=4, r1=8, r2=8)
                    nc.vector.tensor_copy(st_r[:, p8], src)

            nc.scalar.dma_start(out=out_d[hh], in_=st[:, :])

    nc.compile()
    return nc


def _shard_inputs(x, attn_i, w_conv, bn_gamma, bn_beta, bn_mean, bn_var):
    inv_std = (bn_gamma / np.sqrt(bn_var + np.float32(EPS))).astype(np.float32)
    shift = (bn_beta - bn_mean * inv_std).astype(np.float32)
    bias_tile = np.ascontiguousarray(
        np.broadcast_to(shift[None, :], (128, 256))).astype(np.float32)
    w_prep = np.ascontiguousarray((w_conv * inv_std[:, None]).T).astype(np.float32)
    in_maps = []
    for core in range(NCORES):
        b, rh = core // 2, core % 2
        x_sh = x[b, :, 128 * rh:128 * rh + 128, :]
        x_sh = np.ascontiguousarray(
            x_sh.reshape(256, 16, 8, 16, 2, 8).transpose(0, 1, 3, 4, 2, 5)
        ).reshape(256, 16, 2048)
        a_sl = attn_i[1024 * b + 512 * rh: 1024 * b + 512 * rh + 512]
        # [pair, 64win+k, 64nh+q], then partition-major per hh row
        # ([hh, p, pr, 1024]) so each at-load reads 16KiB/partition contiguous
        a_prep = a_sl.reshape(256, 2, 16, 64, 64).transpose(0, 1, 4, 2, 3) \
            .reshape(16, 16, 128, 1024)
        a_prep = np.ascontiguousarray(
            a_prep.transpose(0, 2, 1, 3)).reshape(16, 128, 16384)
        in_maps.append(dict(x_sh=x_sh, at_sh=a_prep, w_prep=w_prep, bias=bias_tile))
    return in_maps


def _unshard_output(results):
    out = np.empty((4, 256, 256, 256), np.float32)
    for core in range(NCORES):
        b, rh = core // 2, core % 2
        raw = results[core]["out_sh"]  # [16, 128, 4096]
        r = raw.reshape(16, 4, 2, 16, 4, 8, 16, 8)  # hh,q,win,d,u,r1,pw,r2
        # ch = 64u+16q+d ; h = 8hh+r1 ; w = 16pw+8win+r2
        oc = r.transpose(4, 1, 3, 0, 5, 6, 2, 7).reshape(256, 128, 256)
        out[b, :, 128 * rh:128 * rh + 128, :] = oc
    return out


def get_program():
    global _cached_nc
    if _cached_nc is None:
        _cached_nc = _build_program()
    return _cached_nc


def run_sharded(in_maps, trace=False, **kwargs):
    nc = get_program()
    return run_bass_kernel_spmd(nc, in_maps, list(range(NCORES)),
                                trace=trace, **kwargs)


def kernel(x, attn_i, w_conv, bn_gamma, bn_beta, bn_mean, bn_var):
    x = np.asarray(x, dtype=np.float32)
    attn_i = np.asarray(attn_i, dtype=np.float32)
    w_conv = np.asarray(w_conv, dtype=np.float32)
    bn_gamma = np.asarray(bn_gamma, dtype=np.float32)
    bn_beta = np.asarray(bn_beta, dtype=np.float32)
    bn_mean = np.asarray(bn_mean, dtype=np.float32)
    bn_var = np.asarray(bn_var, dtype=np.float32)
    in_maps = _shard_inputs(x, attn_i, w_conv, bn_gamma, bn_beta, bn_mean, bn_var)
    res = run_sharded(in_maps)
    return _unshard_output(res.results)



# revision 10
# speedup vs baseline: 2.0187x; 2.0187x over previous
"""TRN2 Bass kernel for nn_ClassAttention (1x1 conv + BN + ReLU + windowed attention).

kernel(**inputs) takes FULL inputs, returns the FULL output [4,256,256,256] f32.
Shards data-parallel over (batch, image-row-half) across 8 NeuronCores, runs a
Bass/Tile SPMD program via run_bass_kernel_spmd, and unshards on the host.

Per-core shard (core = (b, rh) = (core//2, core%2)):
  x_sh   [256c, 16hh, 2048]   x[b,:,128rh:+128,:] rearranged window-contiguous:
                              [c, hh, (pw, win, r1, r2)]
  at_sh  [16hh, 128, 16384]   attn pre-transposed [pair, 64*win+k, 64*nh+q],
                              stored partition-major per row of windows
  w_prep [256c, 256o]         (w_conv * inv_std[:,None]).T  (BN scale folded)
  bias   [128, 256]           (beta - mean*inv_std) broadcast over partitions
  out    [16hh, 128p, 4096]   raw staging dump; host decodes
                              p = 32q+16win+d, f = u*1024+r1*128+pw*8+r2,
                              ch = 64u+16q+d

On-chip pipeline per window-pair (2 windows of 64 pixels, pixels on partitions):
  conv (PE): psum[128pix=(win,r1,r2), 256ch] = x_pair.T @ w_prep
             2 matmuls (K=128 halves), M=128, N=256, fp32
  bias (DVE): tmp = psum + bias_tile
  relu (ACT): block-diagonal V [128, (nh,win,d)]: diag cells = relu(tmp),
              off-diag cells stay zero (zeroed once at start, never rewritten)
  attn (PE): per head nh: one matmul computes BOTH windows via block-diag V:
             out[32,64] = V[:,32nh:+32].T @ At[:,64nh:+64], K=128, N=64,
             tile_position=(0, 32*(nh%4)) -> 4 column-tiles packed in the array
  evac (DVE): psum [128,(u,r1,r2)] -> staging [128, 4096]
  store (ACT hwdge ring): staging -> DRAM, 2 MiB contiguous per row of windows
"""

import numpy as np
from contextlib import ExitStack

import concourse.bacc as bacc
import concourse.tile as tile
import concourse.mybir as mybir
from concourse.bass_utils import run_bass_kernel_spmd

F32 = mybir.dt.float32
F16 = mybir.dt.float16
RELU = mybir.ActivationFunctionType.Relu

EPS = 1e-5
NCORES = 8

_cached_nc = None


def _build_program(n_vbd=10, at_bufs=3, G=4):
    nc = bacc.Bacc("TRN2", target_bir_lowering=False, debug=False)

    x_d = nc.dram_tensor("x_sh", [256, 16, 2048], F16, kind="ExternalInput")
    at_d = nc.dram_tensor("at_sh", [16, 128, 16384], F16, kind="ExternalInput")
    wc_d = nc.dram_tensor("w_prep", [256, 256], F16, kind="ExternalInput")
    b_d = nc.dram_tensor("bias", [128, 256], F32, kind="ExternalInput")
    out_d = nc.dram_tensor("out_sh", [16, 128, 4096], F16, kind="ExternalOutput")

    ngroups = 16 // G

    with tile.TileContext(nc) as tc, ExitStack() as ctx:
        const = ctx.enter_context(tc.tile_pool(name="const", bufs=1))
        xp = ctx.enter_context(tc.tile_pool(name="xp", bufs=2))
        atp = ctx.enter_context(tc.tile_pool(name="atp", bufs=at_bufs))
        vbdp = ctx.enter_context(tc.tile_pool(name="vbdp", bufs=1))
        tvp = ctx.enter_context(tc.tile_pool(name="tvp", bufs=4))
        stp = ctx.enter_context(tc.tile_pool(name="stp", bufs=2))
        pscp = ctx.enter_context(tc.tile_pool(name="pscp", bufs=2, space="PSUM"))
        psap = ctx.enter_context(tc.tile_pool(name="psap", bufs=4, space="PSUM"))

        w0 = const.tile([128, 256], F16, name="w0")
        w1 = const.tile([128, 256], F16, name="w1")
        nc.sync.dma_start(out=w0, in_=wc_d[0:128, :])
        nc.sync.dma_start(out=w1, in_=wc_d[128:256, :])
        bias = const.tile([128, 256], F32, name="bias_t")
        nc.sync.dma_start(out=bias, in_=b_d[:, :])

        # Block-diagonal V tiles: columns = (nh 16, win 2, d 16). Zeroed once;
        # the relu writes only the diagonal cells (win0 -> rows 0:64 of win-0
        # columns, win1 -> rows 64:128 of win-1 columns), so the zeros persist
        # across reuse and each V[:, 32nh:+32] is exactly block-diag(V0, V1).
        vbd = []
        for i in range(n_vbd):
            t = vbdp.tile([128, 512], F16, tag=f"vbd{i}", name=f"vbd{i}")
            nc.vector.memset(t, 0.0)
            vbd.append(t)
        vbd_i = 0

        for hh in range(16):
            xt0 = xp.tile([128, 2048], F16, tag="xt0", name=f"xt0_{hh}")
            xt1 = xp.tile([128, 2048], F16, tag="xt1", name=f"xt1_{hh}")
            nc.sync.dma_start(out=xt0, in_=x_d[0:128, hh, :])
            nc.sync.dma_start(out=xt1, in_=x_d[128:256, hh, :])

            st = stp.tile([128, 4096], F16, tag="st", name=f"st_{hh}")
            # f = u*1024 + r1*128 + pw*8 + r2
            st_r = st.rearrange("p (u r1 pw r2) -> p pw u r1 r2",
                                u=4, r1=8, pw=16, r2=8)

            for g in range(ngroups):
                at = atp.tile([128, 1024 * G], F16, tag="at", name=f"at_{hh}_{g}")
                nc.sync.dma_start(
                    out=at,
                    in_=at_d[hh, :, 1024 * G * g: 1024 * G * (g + 1)])

                Vg = []
                for iG in range(G):
                    p8 = G * g + iG
                    ps = pscp.tile([128, 256], F32, tag="psc", name=f"ps_{hh}_{p8}")
                    xsl = slice(128 * p8, 128 * p8 + 128)
                    nc.tensor.matmul(ps, xt0[:, xsl], w0, start=True, stop=False)
                    nc.tensor.matmul(ps, xt1[:, xsl], w1, start=False, stop=True)
                    tv = tvp.tile([128, 256], F32, tag="tv", name=f"tv_{hh}_{p8}")
                    nc.vector.tensor_add(tv, ps, bias)
                    V = vbd[vbd_i % n_vbd]
                    vbd_i += 1
                    Vr = V.rearrange("p (nh two d) -> p nh two d", nh=16, two=2, d=16)
                    tvr = tv.rearrange("p (a b) -> p a b", a=16)
                    nc.scalar.activation(Vr[0:64, :, 0, :], tvr[0:64], RELU)
                    nc.scalar.activation(Vr[64:128, :, 1, :], tvr[64:128], RELU)
                    Vg.append(V)

                for iG in range(G):
                    p8 = G * g + iG
                    V = Vg[iG]
                    pa = psap.tile([128, 256], F32, tag="pa", name=f"pa_{hh}_{p8}")
                    for j in range(4):
                        for quad in range(4):
                            nh = 4 * j + quad
                            nc.tensor.matmul(
                                pa[32 * quad:32 * quad + 32, 64 * j:64 * j + 64],
                                V[:, 32 * nh:32 * nh + 32],
                                at[:, 1024 * iG + 64 * nh: 1024 * iG + 64 * nh + 64],
                                start=True, stop=True,
                                tile_position=(0, 32 * quad))
                    src = pa.rearrange("p (u r1 r2) -> p u r1 r2", u=4, r1=8, r2=8)
                    nc.vector.tensor_copy(st_r[:, p8], src)

            nc.scalar.dma_start(out=out_d[hh], in_=st[:, :])

    nc.compile()
    return nc


def _shard_inputs(x, attn_i, w_conv, bn_gamma, bn_beta, bn_mean, bn_var):
    inv_std = (bn_gamma / np.sqrt(bn_var + np.float32(EPS))).astype(np.float32)
    shift = (bn_beta - bn_mean * inv_std).astype(np.float32)
    bias_tile = np.ascontiguousarray(
        np.broadcast_to(shift[None, :], (128, 256))).astype(np.float32)
    w_prep = np.ascontiguousarray(
        (w_conv * inv_std[:, None]).T).astype(np.float16)
    x16 = x.astype(np.float16)
    at16 = attn_i.astype(np.float16)
    in_maps = []
    for core in range(NCORES):
        b, rh = core // 2, core % 2
        x_sh = x16[b, :, 128 * rh:128 * rh + 128, :]
        x_sh = np.ascontiguousarray(
            x_sh.reshape(256, 16, 8, 16, 2, 8).transpose(0, 1, 3, 4, 2, 5)
        ).reshape(256, 16, 2048)
        a_sl = at16[1024 * b + 512 * rh: 1024 * b + 512 * rh + 512]
        # [pair, 64win+k, 64nh+q], then partition-major per hh row
        # ([hh, p, pr, 1024]) so each at-load reads 8KiB/partition contiguous
        a_prep = a_sl.reshape(256, 2, 16, 64, 64).transpose(0, 1, 4, 2, 3) \
            .reshape(16, 16, 128, 1024)
        a_prep = np.ascontiguousarray(
            a_prep.transpose(0, 2, 1, 3)).reshape(16, 128, 16384)
        in_maps.append(dict(x_sh=x_sh, at_sh=a_prep, w_prep=w_prep, bias=bias_tile))
    return in_maps


def _unshard_output(results):
    out = np.empty((4, 256, 256, 256), np.float32)
    for core in range(NCORES):
        b, rh = core // 2, core % 2
        raw = np.asarray(results[core]["out_sh"], np.float32)  # [16, 128, 4096]
        r = raw.reshape(16, 4, 2, 16, 4, 8, 16, 8)  # hh,q,win,d,u,r1,pw,r2
        # ch = 64u+16q+d ; h = 8hh+r1 ; w = 16pw+8win+r2
        oc = r.transpose(4, 1, 3, 0, 5, 6, 2, 7).reshape(256, 128, 256)
        out[b, :, 128 * rh:128 * rh + 128, :] = oc
    return out


def get_program():
    global _cached_nc
    if _cached_nc is None:
        _cached_nc = _build_program()
    return _cached_nc


def run_sharded(in_maps, trace=False, **kwargs):
    nc = get_program()
    return run_bass_kernel_spmd(nc, in_maps, list(range(NCORES)),
                                trace=trace, **kwargs)


def kernel(x, attn_i, w_conv, bn_gamma, bn_beta, bn_mean, bn_var):
    x = np.asarray(x, dtype=np.float32)
    attn_i = np.asarray(attn_i, dtype=np.float32)
    w_conv = np.asarray(w_conv, dtype=np.float32)
    bn_gamma = np.asarray(bn_gamma, dtype=np.float32)
    bn_beta = np.asarray(bn_beta, dtype=np.float32)
    bn_mean = np.asarray(bn_mean, dtype=np.float32)
    bn_var = np.asarray(bn_var, dtype=np.float32)
    in_maps = _shard_inputs(x, attn_i, w_conv, bn_gamma, bn_beta, bn_mean, bn_var)
    res = run_sharded(in_maps)
    return _unshard_output(res.results)



# revision 16
# speedup vs baseline: 2.1833x; 1.0815x over previous
"""TRN2 Bass kernel for nn_ClassAttention (1x1 conv + BN + ReLU + windowed attention).

kernel(**inputs) takes FULL inputs, returns the FULL output [4,256,256,256] f32.
Shards data-parallel over (batch, image-row-half) across 8 NeuronCores, runs a
Bass/Tile SPMD program via run_bass_kernel_spmd, and unshards on the host.

Per-core shard (core = (b, rh) = (core//2, core%2)):
  x_sh   [256c, 16hh, 2048]   x[b,:,128rh:+128,:] rearranged window-contiguous:
                              [c, hh, (pw, win, r1, r2)]
  at_sh  [16hh, 128, 16384]   attn pre-transposed [pair, 64*win+k, 64*nh+q],
                              stored partition-major per row of windows
  w_prep [256c, 256o]         (w_conv * inv_std[:,None]).T  (BN scale folded)
  bias   [128, 256]           (beta - mean*inv_std) broadcast over partitions
  out    [16hh, 128p, 4096]   raw staging dump; host decodes
                              p = 32q+16win+d, f = u*1024+r1*128+pw*8+r2,
                              ch = 64u+16q+d

On-chip pipeline per window-pair (2 windows of 64 pixels, pixels on partitions):
  conv (PE): psum[128pix=(win,r1,r2), 256ch] = x_pair.T @ w_prep
             2 matmuls (K=128 halves), M=128, N=256, fp32
  bias (DVE): tmp = psum + bias_tile
  relu (ACT): block-diagonal V [128, (nh,win,d)]: diag cells = relu(tmp),
              off-diag cells stay zero (zeroed once at start, never rewritten)
  attn (PE): per head nh: one matmul computes BOTH windows via block-diag V:
             out[32,64] = V[:,32nh:+32].T @ At[:,64nh:+64], K=128, N=64,
             tile_position=(0, 32*(nh%4)) -> 4 column-tiles packed in the array
  evac (DVE): psum [128,(u,r1,r2)] -> staging [128, 4096]
  store (ACT hwdge ring): staging -> DRAM, 2 MiB contiguous per row of windows
"""

import numpy as np
from contextlib import ExitStack

import concourse.bacc as bacc
import concourse.tile as tile
import concourse.mybir as mybir
from concourse.bass_utils import run_bass_kernel_spmd

F32 = mybir.dt.float32
F16 = mybir.dt.float16
RELU = mybir.ActivationFunctionType.Relu

EPS = 1e-5
NCORES = 8

_cached_nc = None


def _build_program(n_vbd=10, at_bufs=3, G=8):
    nc = bacc.Bacc("TRN2", target_bir_lowering=False, debug=False)

    x_d = nc.dram_tensor("x_sh", [128, 16, 4096], F16, kind="ExternalInput")
    at_d = nc.dram_tensor("at_sh", [16, 128, 16384], F16, kind="ExternalInput")
    wc_d = nc.dram_tensor("w_prep", [256, 256], F16, kind="ExternalInput")
    b_d = nc.dram_tensor("bias", [128, 256], F32, kind="ExternalInput")
    out_d = nc.dram_tensor("out_sh", [16, 128, 4096], F16, kind="ExternalOutput")

    ngroups = 16 // G

    with tile.TileContext(nc) as tc, ExitStack() as ctx:
        const = ctx.enter_context(tc.tile_pool(name="const", bufs=1))
        xp = ctx.enter_context(tc.tile_pool(name="xp", bufs=3))
        atp = ctx.enter_context(tc.tile_pool(name="atp", bufs=at_bufs))
        vbdp = ctx.enter_context(tc.tile_pool(name="vbdp", bufs=1))
        tvp = ctx.enter_context(tc.tile_pool(name="tvp", bufs=4))
        stp = ctx.enter_context(tc.tile_pool(name="stp", bufs=3))
        pscp = ctx.enter_context(tc.tile_pool(name="pscp", bufs=2, space="PSUM"))
        psap = ctx.enter_context(tc.tile_pool(name="psap", bufs=6, space="PSUM"))

        w0 = const.tile([128, 256], F16, name="w0")
        w1 = const.tile([128, 256], F16, name="w1")
        nc.sync.dma_start(out=w0, in_=wc_d[0:128, :])
        nc.sync.dma_start(out=w1, in_=wc_d[128:256, :])
        bias = const.tile([128, 256], F32, name="bias_t")
        nc.sync.dma_start(out=bias, in_=b_d[:, :])

        # Block-diagonal V tiles: columns = (nh 16, win 2, d 16). Zeroed once;
        # the relu writes only the diagonal cells (win0 -> rows 0:64 of win-0
        # columns, win1 -> rows 64:128 of win-1 columns), so the zeros persist
        # across reuse and each V[:, 32nh:+32] is exactly block-diag(V0, V1).
        vbd = []
        for i in range(n_vbd):
            t = vbdp.tile([128, 512], F16, tag=f"vbd{i}", name=f"vbd{i}")
            nc.vector.memset(t, 0.0)
            vbd.append(t)
        vbd_i = 0

        for hh in range(16):
            xt = xp.tile([128, 4096], F16, tag="xt", name=f"xt_{hh}")
            nc.sync.dma_start(out=xt, in_=x_d[:, hh, :])

            st = stp.tile([128, 4096], F16, tag="st", name=f"st_{hh}")
            # f = u*1024 + r1*128 + pw*8 + r2
            st_r = st.rearrange("p (u r1 pw r2) -> p pw u r1 r2",
                                u=4, r1=8, pw=16, r2=8)

            for g in range(ngroups):
                at = atp.tile([128, 1024 * G], F16, tag="at", name=f"at_{hh}_{g}")
                nc.sync.dma_start(
                    out=at,
                    in_=at_d[hh, :, 1024 * G * g: 1024 * G * (g + 1)])

                Vg = []
                for iG in range(G):
                    p8 = G * g + iG
                    ps = pscp.tile([128, 256], F32, tag="psc", name=f"ps_{hh}_{p8}")
                    xsl0 = slice(128 * p8, 128 * p8 + 128)
                    xsl1 = slice(2048 + 128 * p8, 2048 + 128 * p8 + 128)
                    nc.tensor.matmul(ps, xt[:, xsl0], w0, start=True, stop=False)
                    nc.tensor.matmul(ps, xt[:, xsl1], w1, start=False, stop=True)
                    tv = tvp.tile([128, 256], F32, tag="tv", name=f"tv_{hh}_{p8}")
                    nc.vector.tensor_add(tv, ps, bias)
                    V = vbd[vbd_i % n_vbd]
                    vbd_i += 1
                    Vr = V.rearrange("p (nh two d) -> p nh two d", nh=16, two=2, d=16)
                    tvr = tv.rearrange("p (a b) -> p a b", a=16)
                    nc.scalar.activation(Vr[0:64, :, 0, :], tvr[0:64], RELU)
                    nc.scalar.activation(Vr[64:128, :, 1, :], tvr[64:128], RELU)
                    Vg.append(V)

                for iG in range(G):
                    p8 = G * g + iG
                    V = Vg[iG]
                    pa = psap.tile([128, 256], F32, tag="pa", name=f"pa_{hh}_{p8}")
                    for j in range(4):
                        for quad in range(4):
                            nh = 4 * j + quad
                            nc.tensor.matmul(
                                pa[32 * quad:32 * quad + 32, 64 * j:64 * j + 64],
                                V[:, 32 * nh:32 * nh + 32],
                                at[:, 1024 * iG + 64 * nh: 1024 * iG + 64 * nh + 64],
                                start=True, stop=True,
                                tile_position=(0, 32 * quad))
                    src = pa.rearrange("p (u r1 r2) -> p u r1 r2", u=4, r1=8, r2=8)
                    nc.vector.tensor_copy(st_r[:, p8], src)

            nc.scalar.dma_start(out=out_d[hh], in_=st[:, :])

    nc.compile()
    return nc


def _shard_inputs(x, attn_i, w_conv, bn_gamma, bn_beta, bn_mean, bn_var):
    inv_std = (bn_gamma / np.sqrt(bn_var + np.float32(EPS))).astype(np.float32)
    shift = (bn_beta - bn_mean * inv_std).astype(np.float32)
    bias_tile = np.ascontiguousarray(
        np.broadcast_to(shift[None, :], (128, 256))).astype(np.float32)
    w_prep = np.ascontiguousarray(
        (w_conv * inv_std[:, None]).T).astype(np.float16)
    x16 = x.astype(np.float16)
    at16 = attn_i.astype(np.float16)
    in_maps = []
    for core in range(NCORES):
        b, rh = core // 2, core % 2
        x_sh = x16[b, :, 128 * rh:128 * rh + 128, :]
        # [c, hh, (pw, win, r1, r2)] then split c -> (half, cl) and move half
        # into the free axis: [cl, hh, (half, pw, win, r1, r2)]
        x_sh = np.ascontiguousarray(
            x_sh.reshape(2, 128, 16, 8, 16, 2, 8).transpose(1, 2, 0, 4, 5, 3, 6)
        ).reshape(128, 16, 4096)
        a_sl = at16[1024 * b + 512 * rh: 1024 * b + 512 * rh + 512]
        # [pair, 64win+k, 64nh+q], then partition-major per hh row
        # ([hh, p, pr, 1024]) so each at-load reads 8KiB/partition contiguous
        a_prep = a_sl.reshape(256, 2, 16, 64, 64).transpose(0, 1, 4, 2, 3) \
            .reshape(16, 16, 128, 1024)
        a_prep = np.ascontiguousarray(
            a_prep.transpose(0, 2, 1, 3)).reshape(16, 128, 16384)
        in_maps.append(dict(x_sh=x_sh, at_sh=a_prep, w_prep=w_prep, bias=bias_tile))
    return in_maps


def _unshard_output(results):
    out = np.empty((4, 256, 256, 256), np.float32)
    for core in range(NCORES):
        b, rh = core // 2, core % 2
        raw = np.asarray(results[core]["out_sh"], np.float32)  # [16, 128, 4096]
        r = raw.reshape(16, 4, 2, 16, 4, 8, 16, 8)  # hh,q,win,d,u,r1,pw,r2
        # ch = 64u+16q+d ; h = 8hh+r1 ; w = 16pw+8win+r2
        oc = r.transpose(4, 1, 3, 0, 5, 6, 2, 7).reshape(256, 128, 256)
        out[b, :, 128 * rh:128 * rh + 128, :] = oc
    return out


def get_program():
    global _cached_nc
    if _cached_nc is None:
        _cached_nc = _build_program()
    return _cached_nc


def run_sharded(in_maps, trace=False, **kwargs):
    nc = get_program()
    return run_bass_kernel_spmd(nc, in_maps, list(range(NCORES)),
                                trace=trace, **kwargs)


def kernel(x, attn_i, w_conv, bn_gamma, bn_beta, bn_mean, bn_var):
    x = np.asarray(x, dtype=np.float32)
    attn_i = np.asarray(attn_i, dtype=np.float32)
    w_conv = np.asarray(w_conv, dtype=np.float32)
    bn_gamma = np.asarray(bn_gamma, dtype=np.float32)
    bn_beta = np.asarray(bn_beta, dtype=np.float32)
    bn_mean = np.asarray(bn_mean, dtype=np.float32)
    bn_var = np.asarray(bn_var, dtype=np.float32)
    in_maps = _shard_inputs(x, attn_i, w_conv, bn_gamma, bn_beta, bn_mean, bn_var)
    res = run_sharded(in_maps)
    return _unshard_output(res.results)



# revision 21
# speedup vs baseline: 2.1850x; 1.0008x over previous
"""TRN2 Bass kernel for nn_ClassAttention (1x1 conv + BN + ReLU + windowed attention).

kernel(**inputs) takes FULL inputs, returns the FULL output [4,256,256,256] f32.
Shards data-parallel over (batch, image-row-half) across 8 NeuronCores, runs a
Bass/Tile SPMD program via run_bass_kernel_spmd, and unshards on the host.

Per-core shard (core = (b, rh) = (core//2, core%2)):
  x_sh   [256c, 16hh, 2048]   x[b,:,128rh:+128,:] rearranged window-contiguous:
                              [c, hh, (pw, win, r1, r2)]
  at_sh  [16hh, 128, 16384]   attn pre-transposed [pair, 64*win+k, 64*nh+q],
                              stored partition-major per row of windows
  w_prep [256c, 256o]         (w_conv * inv_std[:,None]).T  (BN scale folded)
  bias   [128, 256]           (beta - mean*inv_std) broadcast over partitions
  out    [16hh, 128p, 4096]   raw staging dump; host decodes
                              p = 32q+16win+d, f = u*1024+r1*128+pw*8+r2,
                              ch = 64u+16q+d

On-chip pipeline per window-pair (2 windows of 64 pixels, pixels on partitions):
  conv (PE): psum[128pix=(win,r1,r2), 256ch] = x_pair.T @ w_prep
             2 matmuls (K=128 halves), M=128, N=256, fp32
  bias (DVE): tmp = psum + bias_tile
  relu (ACT): block-diagonal V [128, (nh,win,d)]: diag cells = relu(tmp),
              off-diag cells stay zero (zeroed once at start, never rewritten)
  attn (PE): per head nh: one matmul computes BOTH windows via block-diag V:
             out[32,64] = V[:,32nh:+32].T @ At[:,64nh:+64], K=128, N=64,
             tile_position=(0, 32*(nh%4)) -> 4 column-tiles packed in the array
  evac (DVE): psum [128,(u,r1,r2)] -> staging [128, 4096]
  store (ACT hwdge ring): staging -> DRAM, 2 MiB contiguous per row of windows
"""

import numpy as np
from contextlib import ExitStack

import concourse.bacc as bacc
import concourse.tile as tile
import concourse.mybir as mybir
from concourse.bass_utils import run_bass_kernel_spmd

F32 = mybir.dt.float32
F16 = mybir.dt.float16
RELU = mybir.ActivationFunctionType.Relu

EPS = 1e-5
NCORES = 8

_cached_nc = None


def _build_program(n_vbd=6, at_bufs=3, G=8):
    nc = bacc.Bacc("TRN2", target_bir_lowering=False, debug=False)

    x_d = nc.dram_tensor("x_sh", [128, 16, 4096], F16, kind="ExternalInput")
    at_d = nc.dram_tensor("at_sh", [16, 128, 16384], F16, kind="ExternalInput")
    wc_d = nc.dram_tensor("w_prep", [256, 256], F16, kind="ExternalInput")
    b_d = nc.dram_tensor("bias", [128, 1024], F32, kind="ExternalInput")
    out_d = nc.dram_tensor("out_sh", [16, 128, 4096], F16, kind="ExternalOutput")

    ngroups = 16 // G        # at tiles per hh row
    GB = 4                   # pairs per elementwise batch group
    nbatch = G // GB         # batch groups per at tile

    with tile.TileContext(nc) as tc, ExitStack() as ctx:
        const = ctx.enter_context(tc.tile_pool(name="const", bufs=1))
        xp = ctx.enter_context(tc.tile_pool(name="xp", bufs=3))
        atp = ctx.enter_context(tc.tile_pool(name="atp", bufs=at_bufs))
        vbdp = ctx.enter_context(tc.tile_pool(name="vbdp", bufs=1))
        tvp = ctx.enter_context(tc.tile_pool(name="tvp", bufs=4))
        stp = ctx.enter_context(tc.tile_pool(name="stp", bufs=3))
        pscp = ctx.enter_context(tc.tile_pool(name="pscp", bufs=2, space="PSUM"))
        psap = ctx.enter_context(tc.tile_pool(name="psap", bufs=2, space="PSUM"))

        w0 = const.tile([128, 256], F16, name="w0")
        w1 = const.tile([128, 256], F16, name="w1")
        nc.sync.dma_start(out=w0, in_=wc_d[0:128, :])
        nc.sync.dma_start(out=w1, in_=wc_d[128:256, :])
        bias = const.tile([128, 1024], F32, name="bias_t")
        nc.sync.dma_start(out=bias, in_=b_d[:, :])

        # Block-diagonal V tiles for GB pairs each: columns =
        # (pair GB, nh 16, win 2, d 16). Zeroed once; the relu writes only the
        # diagonal cells (win0 -> rows 0:64 of win-0 columns, win1 -> rows
        # 64:128 of win-1 columns), so the zeros persist across reuse and each
        # V[:, 512p+32nh:+32] is exactly block-diag(V0, V1).
        vbd = []
        for i in range(n_vbd):
            t = vbdp.tile([128, 512 * GB], F16, tag=f"vbd{i}", name=f"vbd{i}")
            nc.vector.memset(t, 0.0)
            vbd.append(t)
        vbd_i = 0

        for hh in range(16):
            xt = xp.tile([128, 4096], F16, tag="xt", name=f"xt_{hh}")
            nc.sync.dma_start(out=xt, in_=x_d[:, hh, :])

            st = stp.tile([128, 4096], F16, tag="st", name=f"st_{hh}")

            for g in range(ngroups):
                at = atp.tile([128, 1024 * G], F16, tag="at", name=f"at_{hh}_{g}")
                nc.sync.dma_start(
                    out=at,
                    in_=at_d[hh, :, 1024 * G * g: 1024 * G * (g + 1)])

                for bg in range(nbatch):
                    grp = nbatch * g + bg          # batch-group index in hh
                    ps4 = pscp.tile([128, 256 * GB], F32, tag="ps4",
                                    name=f"ps4_{hh}_{grp}")
                    for p in range(GB):
                        p16 = GB * grp + p         # pair index in hh
                        xsl0 = slice(128 * p16, 128 * p16 + 128)
                        xsl1 = slice(2048 + 128 * p16, 2048 + 128 * p16 + 128)
                        osl = slice(256 * p, 256 * p + 256)
                        nc.tensor.matmul(ps4[:, osl], xt[:, xsl0], w0,
                                         start=True, stop=False)
                        nc.tensor.matmul(ps4[:, osl], xt[:, xsl1], w1,
                                         start=False, stop=True)
                    tv4 = tvp.tile([128, 256 * GB], F16, tag="tv4",
                                   name=f"tv4_{hh}_{grp}")
                    nc.vector.tensor_add(tv4, ps4, bias)
                    V4 = vbd[vbd_i % n_vbd]
                    vbd_i += 1
                    Vr = V4.rearrange("pt (p nh two d) -> pt p nh two d",
                                      p=GB, nh=16, two=2, d=16)
                    tvr = tv4.rearrange("pt (p a b) -> pt p a b", p=GB, a=16)
                    nc.scalar.activation(Vr[0:64, :, :, 0, :], tvr[0:64], RELU)
                    nc.scalar.activation(Vr[64:128, :, :, 1, :], tvr[64:128],
                                         RELU)

                    pa4 = psap.tile([128, 256 * GB], F32, tag="pa4",
                                    name=f"pa4_{hh}_{grp}")
                    for p in range(GB):
                        ploc = GB * bg + p         # pair index in at tile
                        for j in range(4):
                            for quad in range(4):
                                nh = 4 * j + quad
                                nc.tensor.matmul(
                                    pa4[32 * quad:32 * quad + 32,
                                        256 * p + 64 * j:256 * p + 64 * j + 64],
                                    V4[:, 512 * p + 32 * nh:
                                       512 * p + 32 * nh + 32],
                                    at[:, 1024 * ploc + 64 * nh:
                                       1024 * ploc + 64 * nh + 64],
                                    start=True, stop=True,
                                    tile_position=(0, 32 * quad))
                    nc.vector.tensor_copy(
                        st[:, 1024 * grp:1024 * grp + 1024], pa4)

            nc.scalar.dma_start(out=out_d[hh], in_=st[:, :])

    nc.compile()
    return nc


def _shard_inputs(x, attn_i, w_conv, bn_gamma, bn_beta, bn_mean, bn_var):
    inv_std = (bn_gamma / np.sqrt(bn_var + np.float32(EPS))).astype(np.float32)
    shift = (bn_beta - bn_mean * inv_std).astype(np.float32)
    bias_tile = np.ascontiguousarray(
        np.broadcast_to(np.tile(shift, 4)[None, :], (128, 1024))
    ).astype(np.float32)
    w_prep = np.ascontiguousarray(
        (w_conv * inv_std[:, None]).T).astype(np.float16)
    x16 = x.astype(np.float16)
    at16 = attn_i.astype(np.float16)
    in_maps = []
    for core in range(NCORES):
        b, rh = core // 2, core % 2
        x_sh = x16[b, :, 128 * rh:128 * rh + 128, :]
        # [c, hh, (pw, win, r1, r2)] then split c -> (half, cl) and move half
        # into the free axis: [cl, hh, (half, pw, win, r1, r2)]
        x_sh = np.ascontiguousarray(
            x_sh.reshape(2, 128, 16, 8, 16, 2, 8).transpose(1, 2, 0, 4, 5, 3, 6)
        ).reshape(128, 16, 4096)
        a_sl = at16[1024 * b + 512 * rh: 1024 * b + 512 * rh + 512]
        # [pair, 64win+k, 64nh+q], then partition-major per hh row
        # ([hh, p, pr, 1024]) so each at-load reads 8KiB/partition contiguous
        a_prep = a_sl.reshape(256, 2, 16, 64, 64).transpose(0, 1, 4, 2, 3) \
            .reshape(16, 16, 128, 1024)
        a_prep = np.ascontiguousarray(
            a_prep.transpose(0, 2, 1, 3)).reshape(16, 128, 16384)
        in_maps.append(dict(x_sh=x_sh, at_sh=a_prep, w_prep=w_prep, bias=bias_tile))
    return in_maps


def _unshard_output(results):
    out = np.empty((4, 256, 256, 256), np.float32)
    for core in range(NCORES):
        b, rh = core // 2, core % 2
        raw = np.asarray(results[core]["out_sh"], np.float32)  # [16, 128, 4096]
        # partition = (quad4, win2, d16); f = pw*256 + j*64 + ws1*8 + ws2
        r = raw.reshape(16, 4, 2, 16, 16, 4, 8, 8)  # hh,quad,win,d,pw,j,ws1,ws2
        # ch = 16*(4j+quad)+d ; h = 8hh+ws1 ; w = 16pw+8win+ws2
        oc = r.transpose(5, 1, 3, 0, 6, 4, 2, 7).reshape(256, 128, 256)
        out[b, :, 128 * rh:128 * rh + 128, :] = oc
    return out


def get_program():
    global _cached_nc
    if _cached_nc is None:
        _cached_nc = _build_program()
    return _cached_nc


def run_sharded(in_maps, trace=False, **kwargs):
    nc = get_program()
    return run_bass_kernel_spmd(nc, in_maps, list(range(NCORES)),
                                trace=trace, **kwargs)


def kernel(x, attn_i, w_conv, bn_gamma, bn_beta, bn_mean, bn_var):
    x = np.asarray(x, dtype=np.float32)
    attn_i = np.asarray(attn_i, dtype=np.float32)
    w_conv = np.asarray(w_conv, dtype=np.float32)
    bn_gamma = np.asarray(bn_gamma, dtype=np.float32)
    bn_beta = np.asarray(bn_beta, dtype=np.float32)
    bn_mean = np.asarray(bn_mean, dtype=np.float32)
    bn_var = np.asarray(bn_var, dtype=np.float32)
    in_maps = _shard_inputs(x, attn_i, w_conv, bn_gamma, bn_beta, bn_mean, bn_var)
    res = run_sharded(in_maps)
    return _unshard_output(res.results)



# revision 24
# speedup vs baseline: 2.2281x; 1.0197x over previous
"""TRN2 Bass kernel for nn_ClassAttention (1x1 conv + BN + ReLU + windowed attention).

kernel(**inputs) takes FULL inputs, returns the FULL output [4,256,256,256] f32.
Shards data-parallel over (batch, image-row-half) across 8 NeuronCores, runs a
Bass/Tile SPMD program via run_bass_kernel_spmd, and unshards on the host.

Per-core shard (core = (b, rh) = (core//2, core%2)):
  x_sh   [256c, 16hh, 2048]   x[b,:,128rh:+128,:] rearranged window-contiguous:
                              [c, hh, (pw, win, r1, r2)]
  at_sh  [16hh, 128, 16384]   attn pre-transposed [pair, 64*win+k, 64*nh+q],
                              stored partition-major per row of windows
  w_prep [256c, 256o]         (w_conv * inv_std[:,None]).T  (BN scale folded)
  bias   [128, 256]           (beta - mean*inv_std) broadcast over partitions
  out    [16hh, 128p, 4096]   raw staging dump; host decodes
                              p = 32q+16win+d, f = u*1024+r1*128+pw*8+r2,
                              ch = 64u+16q+d

On-chip pipeline per window-pair (2 windows of 64 pixels, pixels on partitions):
  conv (PE): psum[128pix=(win,r1,r2), 256ch] = x_pair.T @ w_prep
             2 matmuls (K=128 halves), M=128, N=256, fp32
  bias (DVE): tmp = psum + bias_tile
  relu (ACT): block-diagonal V [128, (nh,win,d)]: diag cells = relu(tmp),
              off-diag cells stay zero (zeroed once at start, never rewritten)
  attn (PE): per head nh: one matmul computes BOTH windows via block-diag V:
             out[32,64] = V[:,32nh:+32].T @ At[:,64nh:+64], K=128, N=64,
             tile_position=(0, 32*(nh%4)) -> 4 column-tiles packed in the array
  evac (DVE): psum [128,(u,r1,r2)] -> staging [128, 4096]
  store (ACT hwdge ring): staging -> DRAM, 2 MiB contiguous per row of windows
"""

import numpy as np
from contextlib import ExitStack

import concourse.bacc as bacc
import concourse.tile as tile
import concourse.mybir as mybir
from concourse.bass_utils import run_bass_kernel_spmd

F32 = mybir.dt.float32
F16 = mybir.dt.float16
RELU = mybir.ActivationFunctionType.Relu

EPS = 1e-5
NCORES = 8

_cached_nc = None


def _build_program(n_vbd=4, at_bufs=6, G=4):
    nc = bacc.Bacc("TRN2", target_bir_lowering=False, debug=False)

    x_d = nc.dram_tensor("x_sh", [128, 16, 4096], F16, kind="ExternalInput")
    at_d = nc.dram_tensor("at_sh", [16, 128, 16384], F16, kind="ExternalInput")
    wc_d = nc.dram_tensor("w_prep", [256, 256], F16, kind="ExternalInput")
    b_d = nc.dram_tensor("bias", [128, 1024], F32, kind="ExternalInput")
    out_d = nc.dram_tensor("out_sh", [16, 128, 4096], F16, kind="ExternalOutput")

    ngroups = 16 // G        # at tiles per hh row
    GB = 4                   # pairs per elementwise batch group
    nbatch = G // GB         # batch groups per at tile

    with tile.TileContext(nc) as tc, ExitStack() as ctx:
        const = ctx.enter_context(tc.tile_pool(name="const", bufs=1))
        xp = ctx.enter_context(tc.tile_pool(name="xp", bufs=3))
        atp = ctx.enter_context(tc.tile_pool(name="atp", bufs=at_bufs))
        vbdp = ctx.enter_context(tc.tile_pool(name="vbdp", bufs=1))
        tvp = ctx.enter_context(tc.tile_pool(name="tvp", bufs=4))
        stp = ctx.enter_context(tc.tile_pool(name="stp", bufs=3))
        pscp = ctx.enter_context(tc.tile_pool(name="pscp", bufs=2, space="PSUM"))
        psap = ctx.enter_context(tc.tile_pool(name="psap", bufs=2, space="PSUM"))

        # const loads go on the scalar HWDGE ring so the sync ring's FIFO
        # starts with the bulk at/x loads immediately
        w0 = const.tile([128, 256], F16, name="w0")
        w1 = const.tile([128, 256], F16, name="w1")
        nc.scalar.dma_start(out=w0, in_=wc_d[0:128, :])
        nc.scalar.dma_start(out=w1, in_=wc_d[128:256, :])
        bias = const.tile([128, 1024], F32, name="bias_t")
        nc.scalar.dma_start(out=bias, in_=b_d[:, :])

        # Block-diagonal V tiles for GB pairs each: columns =
        # (pair GB, nh 16, win 2, d 16). Zeroed once; the relu writes only the
        # diagonal cells (win0 -> rows 0:64 of win-0 columns, win1 -> rows
        # 64:128 of win-1 columns), so the zeros persist across reuse and each
        # V[:, 512p+32nh:+32] is exactly block-diag(V0, V1).
        vbd = []
        for i in range(n_vbd):
            t = vbdp.tile([128, 512 * GB], F16, tag=f"vbd{i}", name=f"vbd{i}")
            nc.vector.memset(t, 0.0)
            vbd.append(t)
        vbd_i = 0

        for hh in range(16):
            xt = xp.tile([128, 4096], F16, tag="xt", name=f"xt_{hh}")
            nc.sync.dma_start(out=xt, in_=x_d[:, hh, :])

            st = stp.tile([128, 4096], F16, tag="st", name=f"st_{hh}")

            for g in range(ngroups):
                at = atp.tile([128, 1024 * G], F16, tag="at", name=f"at_{hh}_{g}")
                nc.sync.dma_start(
                    out=at,
                    in_=at_d[hh, :, 1024 * G * g: 1024 * G * (g + 1)])

                for bg in range(nbatch):
                    grp = nbatch * g + bg          # batch-group index in hh
                    ps4 = pscp.tile([128, 256 * GB], F32, tag="ps4",
                                    name=f"ps4_{hh}_{grp}")
                    for p in range(GB):
                        p16 = GB * grp + p         # pair index in hh
                        xsl0 = slice(128 * p16, 128 * p16 + 128)
                        xsl1 = slice(2048 + 128 * p16, 2048 + 128 * p16 + 128)
                        osl = slice(256 * p, 256 * p + 256)
                        nc.tensor.matmul(ps4[:, osl], xt[:, xsl0], w0,
                                         start=True, stop=False)
                        nc.tensor.matmul(ps4[:, osl], xt[:, xsl1], w1,
                                         start=False, stop=True)
                    tv4 = tvp.tile([128, 256 * GB], F16, tag="tv4",
                                   name=f"tv4_{hh}_{grp}")
                    nc.vector.tensor_add(tv4, ps4, bias)
                    V4 = vbd[vbd_i % n_vbd]
                    vbd_i += 1
                    Vr = V4.rearrange("pt (p nh two d) -> pt p nh two d",
                                      p=GB, nh=16, two=2, d=16)
                    tvr = tv4.rearrange("pt (p a b) -> pt p a b", p=GB, a=16)
                    nc.scalar.activation(Vr[0:64, :, :, 0, :], tvr[0:64], RELU)
                    nc.scalar.activation(Vr[64:128, :, :, 1, :], tvr[64:128],
                                         RELU)

                    pa4 = psap.tile([128, 256 * GB], F32, tag="pa4",
                                    name=f"pa4_{hh}_{grp}")
                    for p in range(GB):
                        ploc = GB * bg + p         # pair index in at tile
                        for j in range(4):
                            for quad in range(4):
                                nh = 4 * j + quad
                                nc.tensor.matmul(
                                    pa4[32 * quad:32 * quad + 32,
                                        256 * p + 64 * j:256 * p + 64 * j + 64],
                                    V4[:, 512 * p + 32 * nh:
                                       512 * p + 32 * nh + 32],
                                    at[:, 1024 * ploc + 64 * nh:
                                       1024 * ploc + 64 * nh + 64],
                                    start=True, stop=True,
                                    tile_position=(0, 32 * quad))
                    nc.vector.tensor_copy(
                        st[:, 1024 * grp:1024 * grp + 1024], pa4)
                    # half-row stores: 1 MiB each, shrinks the final
                    # un-overlapped store tail
                    if grp == 1:
                        nc.scalar.dma_start(out=out_d[hh, :, 0:2048],
                                            in_=st[:, 0:2048])
                    elif grp == 3:
                        nc.scalar.dma_start(out=out_d[hh, :, 2048:4096],
                                            in_=st[:, 2048:4096])

    nc.compile()
    return nc


def _shard_inputs(x, attn_i, w_conv, bn_gamma, bn_beta, bn_mean, bn_var):
    inv_std = (bn_gamma / np.sqrt(bn_var + np.float32(EPS))).astype(np.float32)
    shift = (bn_beta - bn_mean * inv_std).astype(np.float32)
    bias_tile = np.ascontiguousarray(
        np.broadcast_to(np.tile(shift, 4)[None, :], (128, 1024))
    ).astype(np.float32)
    w_prep = np.ascontiguousarray(
        (w_conv * inv_std[:, None]).T).astype(np.float16)
    x16 = x.astype(np.float16)
    at16 = attn_i.astype(np.float16)
    in_maps = []
    for core in range(NCORES):
        b, rh = core // 2, core % 2
        x_sh = x16[b, :, 128 * rh:128 * rh + 128, :]
        # [c, hh, (pw, win, r1, r2)] then split c -> (half, cl) and move half
        # into the free axis: [cl, hh, (half, pw, win, r1, r2)]
        x_sh = np.ascontiguousarray(
            x_sh.reshape(2, 128, 16, 8, 16, 2, 8).transpose(1, 2, 0, 4, 5, 3, 6)
        ).reshape(128, 16, 4096)
        a_sl = at16[1024 * b + 512 * rh: 1024 * b + 512 * rh + 512]
        # [pair, 64win+k, 64nh+q], then partition-major per hh row
        # ([hh, p, pr, 1024]) so each at-load reads 8KiB/partition contiguous
        a_prep = a_sl.reshape(256, 2, 16, 64, 64).transpose(0, 1, 4, 2, 3) \
            .reshape(16, 16, 128, 1024)
        a_prep = np.ascontiguousarray(
            a_prep.transpose(0, 2, 1, 3)).reshape(16, 128, 16384)
        in_maps.append(dict(x_sh=x_sh, at_sh=a_prep, w_prep=w_prep, bias=bias_tile))
    return in_maps


def _unshard_output(results):
    out = np.empty((4, 256, 256, 256), np.float32)
    for core in range(NCORES):
        b, rh = core // 2, core % 2
        raw = np.asarray(results[core]["out_sh"], np.float32)  # [16, 128, 4096]
        # partition = (quad4, win2, d16); f = pw*256 + j*64 + ws1*8 + ws2
        r = raw.reshape(16, 4, 2, 16, 16, 4, 8, 8)  # hh,quad,win,d,pw,j,ws1,ws2
        # ch = 16*(4j+quad)+d ; h = 8hh+ws1 ; w = 16pw+8win+ws2
        oc = r.transpose(5, 1, 3, 0, 6, 4, 2, 7).reshape(256, 128, 256)
        out[b, :, 128 * rh:128 * rh + 128, :] = oc
    return out


def get_program():
    global _cached_nc
    if _cached_nc is None:
        _cached_nc = _build_program()
    return _cached_nc


def run_sharded(in_maps, trace=False, **kwargs):
    nc = get_program()
    return run_bass_kernel_spmd(nc, in_maps, list(range(NCORES)),
                                trace=trace, **kwargs)


def kernel(x, attn_i, w_conv, bn_gamma, bn_beta, bn_mean, bn_var):
    x = np.asarray(x, dtype=np.float32)
    attn_i = np.asarray(attn_i, dtype=np.float32)
    w_conv = np.asarray(w_conv, dtype=np.float32)
    bn_gamma = np.asarray(bn_gamma, dtype=np.float32)
    bn_beta = np.asarray(bn_beta, dtype=np.float32)
    bn_mean = np.asarray(bn_mean, dtype=np.float32)
    bn_var = np.asarray(bn_var, dtype=np.float32)
    in_maps = _shard_inputs(x, attn_i, w_conv, bn_gamma, bn_beta, bn_mean, bn_var)
    res = run_sharded(in_maps)
    return _unshard_output(res.results)

